# revision 2
# baseline (speedup 1.0000x reference)
"""Fused AttentionMemory kernel for Trainium2, 8 NeuronCores (SPMD). v4

Math (reference):
    x    = LayerNorm(dec) * gamma + beta                     [B,T,D]
    q    = x @ W1^T + b1                                     [B,T,D]
    k    = enc @ W2^T + b2                                   [M,D]
    attn = LayerNorm_m(q @ k^T / sqrt(D))                    [B,T,M]
    out  = (attn + mem) * 0.5

Reassociation: q @ k^T = x @ G with G[d,m] = sum_e W1[e,d] kt[e,m]; the
LayerNorm on x folds into per-row scalars applied around a matmul on raw
dec:
    attn_pre[t,m] = rsd_t * [ (dec @ G')[t,m] - mu_t*u[m] + sd_t*w[m] ]
with G' = gamma*G/S, u = colsum(G'), w = (beta@G + b1@kt)/S, S=sqrt(D),
sd_t = 1/rsd_t, so rsd*(sd*w) = w.

v4: NO collectives.  Each core redundantly computes the full kt [D,M]
and G' [D,M] (two extra 8.6-GFLOP GEMMs) so the 8 cores are completely
independent: no cross-core sync point can amplify launch skew.  Weights
stream per-output-tile from host-prearranged 4D layouts; dec^T also
streams per-t-tile, keeping SBUF < 180 KiB/partition.

Sharding: batch-parallel over B=8; everything else replicated.
"""

import numpy as np
import ml_dtypes

import concourse.bass as bass
import concourse.tile as tile
from concourse import mybir, bacc
from concourse.bass_utils import run_bass_kernel_spmd
from concourse.masks import make_identity

N_CORES = 8
B, T, D, M = 8, 2048, 2048, 1024
P = 128
ET = D // P                 # 16 contraction / row chunks
TT = T // P                 # 16 t tiles
NCH = M // 512              # 2 moving-dim chunks of 512
S = float(np.sqrt(D))
EPS = 1e-5
FP32 = mybir.dt.float32
BF16 = mybir.dt.bfloat16

_nc_cache = None


def build_nc(loop_k=None, reps=1):
    nc = bacc.Bacc("TRN2", target_bir_lowering=False, debug=False,
                   num_devices=N_CORES)
    dec = nc.declare_dram_parameter("dec", [T, D], BF16, isOutput=False)
    dectc = nc.declare_dram_parameter("dectc", [P, TT, ET, P], BF16,
                                      isOutput=False)
    memh = nc.declare_dram_parameter("memh", [T, M], BF16, isOutput=False)
    encT = nc.declare_dram_parameter("encT", [D, M], BF16, isOutput=False)
    w2c = nc.declare_dram_parameter("w2c", [P, ET, ET, P], BF16,
                                    isOutput=False)
    w1c = nc.declare_dram_parameter("w1c", [P, ET, ET, P], BF16,
                                    isOutput=False)
    b2t = nc.declare_dram_parameter("b2t", [P, ET], FP32, isOutput=False)
    b1t = nc.declare_dram_parameter("b1t", [P, ET], FP32, isOutput=False)
    bet = nc.declare_dram_parameter("bet", [P, ET], FP32, isOutput=False)
    out = nc.declare_dram_parameter("out", [T, M], FP32, isOutput=True)

    with tile.TileContext(nc) as tc:
        for _rep in range(reps):
            _build(tc, nc, dec, dectc, memh, encT, w2c, w1c, b2t, b1t,
                   bet, out, loop_k)
    nc.compile()
    return nc


def _build(tc, nc, dec, dectc, memh, encT, w2c, w1c, b2t, b1t, bet, out,
           loop_k=None):
    from contextlib import ExitStack
    ctx = ExitStack()

    const = ctx.enter_context(tc.tile_pool(name="const", bufs=1))
    identb = const.tile([P, P], BF16)
    make_identity(nc, identb)
    eps_t = const.tile([P, 1], FP32)
    nc.vector.memset(eps_t, EPS)
    eps4_t = const.tile([P, 1], FP32)
    nc.vector.memset(eps4_t, 4.0 * EPS)

    b2c = const.tile([P, ET], FP32)
    nc.sync.dma_start(out=b2c, in_=b2t[:, :])
    b1c = const.tile([P, ET], FP32)
    nc.sync.dma_start(out=b1c, in_=b1t[:, :])
    bec = const.tile([P, ET], FP32)
    nc.sync.dma_start(out=bec, in_=bet[:, :])
    # stacked lhsT columns: ob_st = [ones | beta'], zb_st = [0 | b1']
    ob_st = const.tile([P, ET, 2], BF16)
    nc.vector.memset(ob_st, 1.0)
    nc.any.tensor_copy(out=ob_st[:, :, 1], in_=bec)
    zb_st = const.tile([P, ET, 2], BF16)
    nc.vector.memset(zb_st, 0.0)
    nc.any.tensor_copy(out=zb_st[:, :, 1], in_=b1c)

    # big-resident tiles (whole kernel)
    big = ctx.enter_context(tc.tile_pool(name="big", bufs=1))
    gp_sb = big.tile([P, ET, M], BF16)          # G' (folded), rhs of main mm
    uw_sb = big.tile([2, M], BF16)              # row0 = u, row1 = w
    rsd_all = big.tile([P, TT], FP32)           # 1/sd per t, column per tile
    nmsd_rows = big.tile([2, T], BF16)          # row0 = -mu, row1 = sd

    mm_psum = ctx.enter_context(tc.tile_pool(name="mm_psum", bufs=2, space="PSUM"))
    tp_psum = ctx.enter_context(tc.tile_pool(name="tp_psum", bufs=2, space="PSUM"))
    row_psum = ctx.enter_context(tc.tile_pool(name="row_psum", bufs=1, space="PSUM"))
    mn_dec = ctx.enter_context(tc.tile_pool(name="mn_dec", bufs=3))
    mn_st = ctx.enter_context(tc.tile_pool(name="mn_st", bufs=4))

    def _ln1_stats(j):
        """LN1 stats for t-tile j: fills rsd_all[:, j], nmsd_rows slices."""
        decf = mn_dec.tile([P, D], BF16, tag="dec")
        nc.scalar.dma_start(out=decf, in_=dec[j * P:(j + 1) * P, :])
        st = mn_st.tile([P, 4, 6], FP32, tag="st")
        dsub = decf.rearrange("p (n f) -> p n f", f=512)
        for sg in range(4):
            nc.vector.bn_stats(out=st[:, sg, :], in_=dsub[:, sg, :])
        mv = mn_st.tile([P, 2], FP32, tag="mv")
        nc.vector.bn_aggr(out=mv, in_=st)
        sd = mn_st.tile([P, 1], FP32, tag="sd")
        nc.scalar.activation(out=sd, in_=mv[:, 1:2],
                             func=mybir.ActivationFunctionType.Sqrt,
                             bias=eps_t, scale=1.0)
        nc.vector.reciprocal(out=rsd_all[:, j:j + 1], in_=sd)
        nmsd = mn_st.tile([P, 2], BF16, tag="nmsd")
        nc.vector.tensor_scalar_mul(out=nmsd[:, 0:1], in0=mv[:, 0:1],
                                    scalar1=-1.0)
        nc.any.tensor_copy(out=nmsd[:, 1:2], in_=sd)
        nmp = tp_psum.tile([2, P], BF16, tag="tp")
        nc.tensor.transpose(nmp, nmsd, identb)
        nc.any.tensor_copy(out=nmsd_rows[:, j * P:(j + 1) * P], in_=nmp)

    # ---------------- pre-phase: full kt, then full G' (no collectives)
    with tc.tile_pool(name="pre", bufs=1) as pre, \
         tc.tile_pool(name="wstr", bufs=3) as wstr:
        encT_sb = pre.tile([P, ET, M], BF16)
        kt_sb = pre.tile([P, ET, M], BF16)
        for k in range(ET):
            nc.scalar.dma_start(out=encT_sb[:, k, :],
                                in_=encT[k * P:(k + 1) * P, :])

        # kt[e,m] = sum_d W2[e,d] enc[m,d] + b2[e]; e-tile j per iteration
        for j in range(ET):
            w2j = wstr.tile([P, ET, P], BF16, tag="w")
            nc.sync.dma_start(out=w2j, in_=w2c[:, j, :, :])
            pk = mm_psum.tile([P, M], FP32, tag="mm")
            for k in range(ET):
                for ch in range(NCH):
                    nc.tensor.matmul(
                        pk[:, ch * 512:(ch + 1) * 512],
                        lhsT=w2j[:, k, :],
                        rhs=encT_sb[:, k, ch * 512:(ch + 1) * 512],
                        start=(k == 0), stop=(k == ET - 1))
            nc.vector.tensor_scalar(out=kt_sb[:, j, :], in0=pk,
                                    scalar1=b2c[:, j:j + 1],
                                    scalar2=None, op0=mybir.AluOpType.add)
            if j % 2 == 0:
                _ln1_stats(j // 2)

        # G'[d,m] = sum_e W1f[e,d] kt[e,m]; d-tile j per iteration
        for j in range(ET):
            w1j = wstr.tile([P, ET, P], BF16, tag="w")
            nc.sync.dma_start(out=w1j, in_=w1c[:, j, :, :])
            pg = mm_psum.tile([P, M], FP32, tag="mm")
            for k in range(ET):
                for ch in range(NCH):
                    nc.tensor.matmul(
                        pg[:, ch * 512:(ch + 1) * 512],
                        lhsT=w1j[:, k, :],
                        rhs=kt_sb[:, k, ch * 512:(ch + 1) * 512],
                        start=(k == 0), stop=(k == ET - 1))
            nc.any.tensor_copy(out=gp_sb[:, j, :], in_=pg)
            if j % 2 == 0:
                _ln1_stats(ET // 2 + j // 2)

        # one rank-2 chain: row0 = ones@G' = u, row1 = b1'@kt + beta'@G' = w
        puw = row_psum.tile([2, M], FP32, tag="row")
        for k in range(ET):
            for ch in range(NCH):
                nc.tensor.matmul(puw[:, ch * 512:(ch + 1) * 512],
                                 lhsT=zb_st[:, k, :],
                                 rhs=kt_sb[:, k, ch * 512:(ch + 1) * 512],
                                 start=(k == 0), stop=False)
        for k in range(ET):
            for ch in range(NCH):
                nc.tensor.matmul(puw[:, ch * 512:(ch + 1) * 512],
                                 lhsT=ob_st[:, k, :],
                                 rhs=gp_sb[:, k, ch * 512:(ch + 1) * 512],
                                 start=False, stop=(k == ET - 1))
        nc.any.tensor_copy(out=uw_sb, in_=puw)

    # ---------------- main loop over t tiles (stats precomputed above)
    with tc.tile_pool(name="mn_io", bufs=3) as mn_io, \
         tc.tile_pool(name="mn_wk", bufs=2) as mn_wk, \
         tc.tile_pool(name="mn_dt", bufs=3) as mn_dt:
      def _main_phase():
          for j in range(TT):
              dtj = mn_dt.tile([P, ET, P], BF16, tag="dt")
              nc.sync.dma_start(out=dtj, in_=dectc[:, j, :, :])
              memf = mn_io.tile([P, M], BF16, tag="mem")
              nc.scalar.dma_start(out=memf, in_=memh[j * P:(j + 1) * P, :])

              pm = mm_psum.tile([P, M], FP32, tag="mm")
              for k in range(ET):
                  for ch in range(NCH):
                      nc.tensor.matmul(pm[:, ch * 512:(ch + 1) * 512],
                                       lhsT=dtj[:, k, :],
                                       rhs=gp_sb[:, k, ch * 512:(ch + 1) * 512],
                                       start=(k == 0), stop=False)
              for ch in range(NCH):
                  nc.tensor.matmul(pm[:, ch * 512:(ch + 1) * 512],
                                   lhsT=nmsd_rows[:, j * P:(j + 1) * P],
                                   rhs=uw_sb[:, ch * 512:(ch + 1) * 512],
                                   start=False, stop=True)

              # fused epilogue.  y = rsd*pm (row scale), so LN2 stats of y
              # derive exactly from stats of pm:
              #   mu2 = rsd*mu_p, var2 = rsd^2*var_p
              #   z = 0.5*LN2(y) = a*(pm - mu_p),
              #   a = 0.5*rsd/sqrt(rsd^2*var_p + EPS)
              st2 = mn_wk.tile([P, 2, 6], FP32, tag="st2")
              for sg in range(2):
                  nc.vector.bn_stats(out=st2[:, sg, :],
                                     in_=pm[:, sg * 512:(sg + 1) * 512])
              mv2 = mn_wk.tile([P, 2], FP32, tag="mv2")
              nc.vector.bn_aggr(out=mv2, in_=st2)
              r2 = mn_wk.tile([P, 1], FP32, tag="r2")
              nc.vector.tensor_mul(out=r2, in0=rsd_all[:, j:j + 1],
                                    in1=rsd_all[:, j:j + 1])
              v2 = mn_wk.tile([P, 1], FP32, tag="v2")
              nc.vector.tensor_mul(out=v2, in0=r2, in1=mv2[:, 1:2])
              a = mn_wk.tile([P, 1], FP32, tag="a")
              nc.scalar.activation(out=a, in_=v2,
                                   func=mybir.ActivationFunctionType.Sqrt,
                                   bias=eps4_t, scale=4.0)
              nc.vector.reciprocal(out=a, in_=a)
              nc.vector.tensor_mul(out=a, in0=a, in1=rsd_all[:, j:j + 1])
              z = mn_wk.tile([P, M], FP32, tag="z")
              nc.vector.tensor_scalar(out=z, in0=pm, scalar1=mv2[:, 0:1],
                                      scalar2=a,
                                      op0=mybir.AluOpType.subtract,
                                      op1=mybir.AluOpType.mult)
              o = mn_io.tile([P, M], FP32, tag="o")
              nc.vector.tensor_add(out=o, in0=z, in1=memf)
              nc.scalar.dma_start(out=out[j * P:(j + 1) * P, :], in_=o)
      if loop_k is None:
          _main_phase()
      else:
          with tc.For_i(0, loop_k, 1) as _i:
              _main_phase()
    ctx.close()


def _reorg(mat):
    """[R, C] -> [P, R//P, C//P, P] with [p, j, k, c] = mat[k*P+p, j*P+c].

    Gives per-j ([:, j, :, :]) contiguous lhsT slices: chunk k =
    mat[k*P:(k+1)*P, j*P:(j+1)*P]."""
    R, C = mat.shape
    return np.ascontiguousarray(
        mat.reshape(R // P, P, C // P, P).transpose(1, 2, 0, 3),
        dtype=ml_dtypes.bfloat16)


def _make_in_maps(dec_output, enc_out_mem_mean, mem_attn_out,
                  ln_gamma, ln_beta, W1, b1, W2, b2):
    f = np.float32
    bf = ml_dtypes.bfloat16
    encT = np.ascontiguousarray(np.asarray(enc_out_mem_mean, dtype=f).T,
                                dtype=bf)
    gamma = np.asarray(ln_gamma, f)
    bet = np.ascontiguousarray((np.asarray(ln_beta, f) / gamma)
                               .reshape(ET, P).T, dtype=f)
    b1t = np.ascontiguousarray((np.asarray(b1, f) / S).reshape(ET, P).T,
                               dtype=f)
    b2t = np.ascontiguousarray(np.asarray(b2, f).reshape(ET, P).T, dtype=f)
    W1f = np.asarray(W1, dtype=f) * (gamma / S)[None, :]   # fold gamma/S
    w1c = _reorg(W1f)                                      # lhsT = W1f[e,d]
    w2c = _reorg(np.asarray(W2, dtype=f).T)                # lhsT = W2^T[d,e]
    in_maps = []
    for i in range(N_CORES):
        deci = np.asarray(dec_output[i], dtype=f)
        in_maps.append({
            "dec": np.ascontiguousarray(deci, dtype=bf),
            "dectc": _reorg(np.ascontiguousarray(deci.T)),
            "memh": np.ascontiguousarray(
                np.asarray(mem_attn_out[i], dtype=f) * 0.5, dtype=bf),
            "encT": encT,
            "w2c": w2c,
            "w1c": w1c,
            "b2t": b2t,
            "b1t": b1t,
            "bet": bet,
        })
    return in_maps


def kernel(**inputs) -> np.ndarray:
    global _nc_cache
    if _nc_cache is None:
        _nc_cache = build_nc()
    in_maps = _make_in_maps(**inputs)
    res = run_bass_kernel_spmd(_nc_cache, in_maps,
                               core_ids=list(range(N_CORES)))
    return np.stack([res.results[i]["out"] for i in range(N_CORES)], axis=0)


# revision 3
# speedup vs baseline: 1.0345x; 1.0345x over previous
"""Fused AttentionMemory kernel for Trainium2, 8 NeuronCores (SPMD). v4

Math (reference):
    x    = LayerNorm(dec) * gamma + beta                     [B,T,D]
    q    = x @ W1^T + b1                                     [B,T,D]
    k    = enc @ W2^T + b2                                   [M,D]
    attn = LayerNorm_m(q @ k^T / sqrt(D))                    [B,T,M]
    out  = (attn + mem) * 0.5

Reassociation: q @ k^T = x @ G with G[d,m] = sum_e W1[e,d] kt[e,m]; the
LayerNorm on x folds into per-row scalars applied around a matmul on raw
dec:
    attn_pre[t,m] = rsd_t * [ (dec @ G')[t,m] - mu_t*u[m] + sd_t*w[m] ]
with G' = gamma*G/S, u = colsum(G'), w = (beta@G + b1@kt)/S, S=sqrt(D),
sd_t = 1/rsd_t, so rsd*(sd*w) = w.

v4: NO collectives.  Each core redundantly computes the full kt [D,M]
and G' [D,M] (two extra 8.6-GFLOP GEMMs) so the 8 cores are completely
independent: no cross-core sync point can amplify launch skew.  Weights
stream per-output-tile from host-prearranged 4D layouts; dec^T also
streams per-t-tile, keeping SBUF < 180 KiB/partition.

Sharding: batch-parallel over B=8; everything else replicated.
"""

import numpy as np
import ml_dtypes

import concourse.bass as bass
import concourse.tile as tile
from concourse import mybir, bacc
from concourse.bass_utils import run_bass_kernel_spmd
from concourse.masks import make_identity

N_CORES = 8
B, T, D, M = 8, 2048, 2048, 1024
P = 128
ET = D // P                 # 16 contraction / row chunks
TT = T // P                 # 16 t tiles
NCH = M // 512              # 2 moving-dim chunks of 512
S = float(np.sqrt(D))
EPS = 1e-5
FP32 = mybir.dt.float32
BF16 = mybir.dt.bfloat16

_nc_cache = None


def build_nc(loop_k=None, reps=1):
    nc = bacc.Bacc("TRN2", target_bir_lowering=False, debug=False,
                   num_devices=1)
    dec = nc.declare_dram_parameter("dec", [T, D], BF16, isOutput=False)
    dectc = nc.declare_dram_parameter("dectc", [P, TT, ET, P], BF16,
                                      isOutput=False)
    memh = nc.declare_dram_parameter("memh", [T, M], BF16, isOutput=False)
    encT = nc.declare_dram_parameter("encT", [D, M], BF16, isOutput=False)
    w2c = nc.declare_dram_parameter("w2c", [P, ET, ET, P], BF16,
                                    isOutput=False)
    w1c = nc.declare_dram_parameter("w1c", [P, ET, ET, P], BF16,
                                    isOutput=False)
    b2t = nc.declare_dram_parameter("b2t", [P, ET], FP32, isOutput=False)
    cbt = nc.declare_dram_parameter("cbt", [P, ET, 2], FP32, isOutput=False)
    out = nc.declare_dram_parameter("out", [T, M], FP32, isOutput=True)

    with tile.TileContext(nc) as tc:
        for _rep in range(reps):
            _build(tc, nc, dec, dectc, memh, encT, w2c, w1c, b2t, cbt,
                   out, loop_k)
    nc.compile()
    return nc


def _build(tc, nc, dec, dectc, memh, encT, w2c, w1c, b2t, cbt, out,
           loop_k=None):
    from contextlib import ExitStack
    ctx = ExitStack()

    const = ctx.enter_context(tc.tile_pool(name="const", bufs=1))
    identb = const.tile([P, P], BF16)
    make_identity(nc, identb)
    eps_t = const.tile([P, 1], FP32)
    nc.vector.memset(eps_t, EPS)
    eps4_t = const.tile([P, 1], FP32)
    nc.vector.memset(eps4_t, 4.0 * EPS)

    # tiny const loads go on the SWDGE (gpsimd) ring so the HWDGE rings
    # start with the critical-path weight/activation loads
    b2c = const.tile([P, ET], FP32)
    nc.gpsimd.dma_start(out=b2c, in_=b2t[:, :])
    cbf = const.tile([P, ET, 2], FP32)
    nc.gpsimd.dma_start(out=cbf, in_=cbt[:, :, :])
    # stacked lhsT columns [c1 | c2+b1'] (host-folded): uw = cb^T @ kt
    cb_st = const.tile([P, ET, 2], BF16)
    nc.any.tensor_copy(out=cb_st, in_=cbf)

    # big-resident tiles (whole kernel)
    big = ctx.enter_context(tc.tile_pool(name="big", bufs=1))
    gp_sb = big.tile([P, ET, M], BF16)          # G' (folded), rhs of main mm
    uw_sb = big.tile([2, M], BF16)              # row0 = u, row1 = w
    rsd_all = big.tile([P, TT], FP32)           # 1/sd per t, column per tile
    nmsd_rows = big.tile([2, T], BF16)          # row0 = -mu, row1 = sd
    # first two main-loop lhsT tiles prefetched into non-recycled SBUF so
    # the main loop doesn't stall on the pre-phase pool release
    dt01 = big.tile([P, 2, ET, P], BF16)
    nc.gpsimd.dma_start(out=dt01[:, 0], in_=dectc[:, 0, :, :])
    nc.gpsimd.dma_start(out=dt01[:, 1], in_=dectc[:, 1, :, :])

    mm_psum = ctx.enter_context(tc.tile_pool(name="mm_psum", bufs=2, space="PSUM"))
    tp_psum = ctx.enter_context(tc.tile_pool(name="tp_psum", bufs=2, space="PSUM"))
    row_psum = ctx.enter_context(tc.tile_pool(name="row_psum", bufs=1, space="PSUM"))
    mn_dec = ctx.enter_context(tc.tile_pool(name="mn_dec", bufs=3))
    mn_st = ctx.enter_context(tc.tile_pool(name="mn_st", bufs=4))

    def _ln1_stats(j):
        """LN1 stats for t-tile j: fills rsd_all[:, j], nmsd_rows slices."""
        decf = mn_dec.tile([P, D], BF16, tag="dec")
        nc.scalar.dma_start(out=decf, in_=dec[j * P:(j + 1) * P, :])
        st = mn_st.tile([P, 4, 6], FP32, tag="st")
        dsub = decf.rearrange("p (n f) -> p n f", f=512)
        for sg in range(4):
            nc.vector.bn_stats(out=st[:, sg, :], in_=dsub[:, sg, :])
        mv = mn_st.tile([P, 2], FP32, tag="mv")
        nc.vector.bn_aggr(out=mv, in_=st)
        sd = mn_st.tile([P, 1], FP32, tag="sd")
        nc.scalar.activation(out=sd, in_=mv[:, 1:2],
                             func=mybir.ActivationFunctionType.Sqrt,
                             bias=eps_t, scale=1.0)
        nc.vector.reciprocal(out=rsd_all[:, j:j + 1], in_=sd)
        nmsd = mn_st.tile([P, 2], BF16, tag="nmsd")
        nc.vector.tensor_scalar_mul(out=nmsd[:, 0:1], in0=mv[:, 0:1],
                                    scalar1=-1.0)
        nc.any.tensor_copy(out=nmsd[:, 1:2], in_=sd)
        nmp = tp_psum.tile([2, P], BF16, tag="tp")
        nc.tensor.transpose(nmp, nmsd, identb)
        nc.any.tensor_copy(out=nmsd_rows[:, j * P:(j + 1) * P], in_=nmp)

    # ---------------- pre-phase: full kt, then full G' (no collectives)
    with tc.tile_pool(name="pre", bufs=1) as pre, \
         tc.tile_pool(name="wstr", bufs=3) as wstr:
        encT_sb = pre.tile([P, ET, M], BF16)
        kt_sb = pre.tile([P, ET, M], BF16)
        for k in range(ET):
            nc.scalar.dma_start(out=encT_sb[:, k, :],
                                in_=encT[k * P:(k + 1) * P, :])

        # kt[e,m] = sum_d W2[e,d] enc[m,d] + b2[e]; e-tile j per iteration
        for j in range(ET):
            w2j = wstr.tile([P, ET, P], BF16, tag="w")
            nc.sync.dma_start(out=w2j, in_=w2c[:, j, :, :])
            pk = mm_psum.tile([P, M], FP32, tag="mm")
            for k in range(ET):
                for ch in range(NCH):
                    nc.tensor.matmul(
                        pk[:, ch * 512:(ch + 1) * 512],
                        lhsT=w2j[:, k, :],
                        rhs=encT_sb[:, k, ch * 512:(ch + 1) * 512],
                        start=(k == 0), stop=(k == ET - 1))
            nc.vector.tensor_scalar(out=kt_sb[:, j, :], in0=pk,
                                    scalar1=b2c[:, j:j + 1],
                                    scalar2=None, op0=mybir.AluOpType.add)
            if j % 2 == 0:
                _ln1_stats(j // 2)

        # G'[d,m] = sum_e W1f[e,d] kt[e,m]; d-tile j per iteration
        for j in range(ET):
            w1j = wstr.tile([P, ET, P], BF16, tag="w")
            nc.sync.dma_start(out=w1j, in_=w1c[:, j, :, :])
            pg = mm_psum.tile([P, M], FP32, tag="mm")
            for k in range(ET):
                for ch in range(NCH):
                    nc.tensor.matmul(
                        pg[:, ch * 512:(ch + 1) * 512],
                        lhsT=w1j[:, k, :],
                        rhs=kt_sb[:, k, ch * 512:(ch + 1) * 512],
                        start=(k == 0), stop=(k == ET - 1))
            nc.any.tensor_copy(out=gp_sb[:, j, :], in_=pg)
            if j % 2 == 0:
                _ln1_stats(ET // 2 + j // 2)

        # one rank-2 chain over kt only: row0 = c1@kt = u, row1 = (c2+b1')@kt
        # = w, with c1 = W1@gamma/S and c2 = W1@beta/S folded host-side
        puw = row_psum.tile([2, M], FP32, tag="row")
        for k in range(ET):
            for ch in range(NCH):
                nc.tensor.matmul(puw[:, ch * 512:(ch + 1) * 512],
                                 lhsT=cb_st[:, k, :],
                                 rhs=kt_sb[:, k, ch * 512:(ch + 1) * 512],
                                 start=(k == 0), stop=(k == ET - 1))
        nc.any.tensor_copy(out=uw_sb, in_=puw)

    # ---------------- main loop over t tiles (stats precomputed above)
    with tc.tile_pool(name="mn_io", bufs=3) as mn_io, \
         tc.tile_pool(name="mn_wk", bufs=2) as mn_wk, \
         tc.tile_pool(name="mn_dt", bufs=3) as mn_dt:
      def _main_phase():
          for j in range(TT):
              if j < 2:
                  dtj = dt01[:, j]
              else:
                  dtj = mn_dt.tile([P, ET, P], BF16, tag="dt")
                  nc.sync.dma_start(out=dtj, in_=dectc[:, j, :, :])
              memf = mn_io.tile([P, M], BF16, tag="mem")
              nc.scalar.dma_start(out=memf, in_=memh[j * P:(j + 1) * P, :])

              pm = mm_psum.tile([P, M], FP32, tag="mm")
              for k in range(ET):
                  for ch in range(NCH):
                      nc.tensor.matmul(pm[:, ch * 512:(ch + 1) * 512],
                                       lhsT=dtj[:, k, :],
                                       rhs=gp_sb[:, k, ch * 512:(ch + 1) * 512],
                                       start=(k == 0), stop=False)
              for ch in range(NCH):
                  nc.tensor.matmul(pm[:, ch * 512:(ch + 1) * 512],
                                   lhsT=nmsd_rows[:, j * P:(j + 1) * P],
                                   rhs=uw_sb[:, ch * 512:(ch + 1) * 512],
                                   start=False, stop=True)

              # fused epilogue.  y = rsd*pm (row scale), so LN2 stats of y
              # derive exactly from stats of pm:
              #   mu2 = rsd*mu_p, var2 = rsd^2*var_p
              #   z = 0.5*LN2(y) = a*(pm - mu_p),
              #   a = 0.5*rsd/sqrt(rsd^2*var_p + EPS)
              st2 = mn_wk.tile([P, 2, 6], FP32, tag="st2")
              for sg in range(2):
                  nc.vector.bn_stats(out=st2[:, sg, :],
                                     in_=pm[:, sg * 512:(sg + 1) * 512])
              mv2 = mn_wk.tile([P, 2], FP32, tag="mv2")
              nc.vector.bn_aggr(out=mv2, in_=st2)
              r2 = mn_wk.tile([P, 1], FP32, tag="r2")
              nc.vector.tensor_mul(out=r2, in0=rsd_all[:, j:j + 1],
                                    in1=rsd_all[:, j:j + 1])
              v2 = mn_wk.tile([P, 1], FP32, tag="v2")
              nc.vector.tensor_mul(out=v2, in0=r2, in1=mv2[:, 1:2])
              a = mn_wk.tile([P, 1], FP32, tag="a")
              nc.scalar.activation(out=a, in_=v2,
                                   func=mybir.ActivationFunctionType.Sqrt,
                                   bias=eps4_t, scale=4.0)
              nc.vector.reciprocal(out=a, in_=a)
              nc.vector.tensor_mul(out=a, in0=a, in1=rsd_all[:, j:j + 1])
              z = mn_wk.tile([P, M], FP32, tag="z")
              nc.vector.tensor_scalar(out=z, in0=pm, scalar1=mv2[:, 0:1],
                                      scalar2=a,
                                      op0=mybir.AluOpType.subtract,
                                      op1=mybir.AluOpType.mult)
              o = mn_io.tile([P, M], FP32, tag="o")
              nc.vector.tensor_add(out=o, in0=z, in1=memf)
              nc.scalar.dma_start(out=out[j * P:(j + 1) * P, :], in_=o)
      if loop_k is None:
          _main_phase()
      else:
          with tc.For_i(0, loop_k, 1) as _i:
              _main_phase()
    ctx.close()


def _reorg(mat):
    """[R, C] -> [P, R//P, C//P, P] with [p, j, k, c] = mat[k*P+p, j*P+c].

    Gives per-j ([:, j, :, :]) contiguous lhsT slices: chunk k =
    mat[k*P:(k+1)*P, j*P:(j+1)*P]."""
    R, C = mat.shape
    return np.ascontiguousarray(
        mat.reshape(R // P, P, C // P, P).transpose(1, 2, 0, 3),
        dtype=ml_dtypes.bfloat16)


def _make_in_maps(dec_output, enc_out_mem_mean, mem_attn_out,
                  ln_gamma, ln_beta, W1, b1, W2, b2):
    f = np.float32
    bf = ml_dtypes.bfloat16
    encT = np.ascontiguousarray(np.asarray(enc_out_mem_mean, dtype=f).T,
                                dtype=bf)
    gamma = np.asarray(ln_gamma, f)
    beta = np.asarray(ln_beta, f)
    b2t = np.ascontiguousarray(np.asarray(b2, f).reshape(ET, P).T, dtype=f)
    W1_ = np.asarray(W1, dtype=f)
    W1f = W1_ * (gamma / S)[None, :]                       # fold gamma/S
    w1c = _reorg(W1f)                                      # lhsT = W1f[e,d]
    w2c = _reorg(np.asarray(W2, dtype=f).T)                # lhsT = W2^T[d,e]
    # coefficient vectors for the u/w rank-2 projection of kt:
    #   u = c1 @ kt,  w = (c2 + b1/S) @ kt
    c1 = (W1_ @ gamma) / S
    c2pb = (W1_ @ beta + np.asarray(b1, f)) / S
    cbt = np.ascontiguousarray(
        np.stack([c1, c2pb], axis=1).reshape(ET, P, 2).transpose(1, 0, 2),
        dtype=f)
    in_maps = []
    for i in range(N_CORES):
        deci = np.asarray(dec_output[i], dtype=f)
        in_maps.append({
            "dec": np.ascontiguousarray(deci, dtype=bf),
            "dectc": _reorg(np.ascontiguousarray(deci.T)),
            "memh": np.ascontiguousarray(
                np.asarray(mem_attn_out[i], dtype=f) * 0.5, dtype=bf),
            "encT": encT,
            "w2c": w2c,
            "w1c": w1c,
            "b2t": b2t,
            "cbt": cbt,
        })
    return in_maps


def kernel(**inputs) -> np.ndarray:
    global _nc_cache
    if _nc_cache is None:
        _nc_cache = build_nc()
    in_maps = _make_in_maps(**inputs)
    res = run_bass_kernel_spmd(_nc_cache, in_maps,
                               core_ids=list(range(N_CORES)))
    return np.stack([res.results[i]["out"] for i in range(N_CORES)], axis=0)
